# revision 32
# baseline (speedup 1.0000x reference)
"""ObjectAttentionBlock Trainium2 Bass kernel — fp8 DoubleRow conv variant.

Same algorithm as the baseline (conv6 folded into the value matrix, all
matmuls fp8 DoubleRow), plus scheduling work derived from the HW trace:
  - PE warmup: ~8 dummy DoubleRow matmuls on a zeroed tile run while the
    first input DMAs are in flight, so the HAM clock is ramped and the
    first real matmul issues as soon as data lands (~10us vs ~17us).
  - Startup DMAs spread across four trigger queues (sync/scalar/vector/
    gpsimd) with the conv0-critical transfers first.
  - x is uploaded in a [kt, p, j, col] host layout so each pixel tile
    loads with 2 DMA triggers instead of 4.
  - Output is bf16 in a [p, o, col] layout: one merged [128, 4, 512]
    DMA per tile on the hw-DGE sync queue (halves write bytes, 1/4 the
    triggers, small end-of-run drain).
  - Single 8-bank PSUM pool (instead of 4+4): reuse distance grows to
    ~8 matmuls, so eviction jitter on ACT/DVE no longer back-pressures
    the PE through PSUM WAR hazards. (GPSIMD cannot read PSUM, so
    evictions stay split 6 ACT : 2 DVE as in the baseline.)
"""

import numpy as np
import ml_dtypes

import concourse.bass as bass
import concourse.mybir as mybir
import concourse.tile as tile
from concourse import bacc, bass_utils

N = 8
C = 512
K = 256
H = 128
W_IMG = 128
HW = H * W_IMG
P = 128          # partition width
CT = C // P      # 4 channel tiles
KT = K // P      # 2 region tiles
DT = 2           # DoubleRow kt groups (C / 256)
T = 512          # pixel tile (matmul moving dim / one PSUM bank of fp32)
NT = HW // T     # 32 pixel tiles
ALPHA = 1.0 / float(np.sqrt(C))
N_WARM = 4       # dummy matmuls to ramp the PE clock during input DMA

f32 = mybir.dt.float32
bf16 = mybir.dt.bfloat16
fp8 = mybir.dt.float8e4
AF = mybir.ActivationFunctionType
DR = mybir.MatmulPerfMode.DoubleRow
NP_BF16 = ml_dtypes.bfloat16
NP_FP8 = mybir.dt.np(fp8)


def build_module(n_tiles=NT):
    """Build and compile the per-core Bass module (SPMD: same on all cores)."""
    nc = bacc.Bacc("TRN2", target_bir_lowering=False, debug=False)
    # x in [kt, p, j, col] layout: row kt*256 + j*128 + p of the [C, HW] map
    xin = nc.dram_tensor("xin", [DT, P, 2, HW], fp8, kind="ExternalInput").ap()
    # proxy twice: DoubleRow fp8 layout for the key path (softmax absorbs
    # the fp8 noise there), plain bf16 for the value path (fp8 there leaks
    # straight into the output and blows the error budget)
    pin_dr = nc.dram_tensor("pindr", [DT, P, 2, K], fp8, kind="ExternalInput").ap()
    pin = nc.dram_tensor("pin", [C, K], bf16, kind="ExternalInput").ap()
    # conv layers 0-3 as DoubleRow-interleaved fp8 weights
    wdr_d = nc.dram_tensor("wdr", [4, DT, P, 2, C], fp8, kind="ExternalInput").ap()
    # value-path conv layers 4,5,6 stay bf16
    wt = nc.dram_tensor("wt", [3, C, C], bf16, kind="ExternalInput").ap()
    sbc_d = nc.dram_tensor("sbc", [P, 28], f32, kind="ExternalInput").ap()
    b6bc_d = nc.dram_tensor("b6bc", [P, C], f32, kind="ExternalInput").ap()
    onesdr_d = nc.dram_tensor("onesdr", [P, 2, P], fp8, kind="ExternalInput").ap()
    # out in [p, o, col] layout: row o*128 + p of the [C, HW] map, bf16
    out_d = nc.dram_tensor("out", [P, CT, HW], bf16, kind="ExternalOutput").ap()

    with tile.TileContext(nc) as tc:
        with (
            tc.tile_pool(name="const", bufs=1) as cpool,
            tc.tile_pool(name="loop", bufs=2) as lpool,
            tc.tile_pool(name="ps8", bufs=8, space="PSUM") as psp,
        ):
            psa = psp
            # ---- PE warmup + constants ----
            # The HAM clock-gates an idle PE; dummy DoubleRow matmuls on a
            # zeroed tile ramp it to full rate while the first input DMAs
            # are in flight.
            zwarm = cpool.tile([P, 2, T], fp8, name="zwarm")
            nc.gpsimd.memset(zwarm[:], 0)
            for wmi in range(N_WARM):
                pw = psp.tile([P, T], f32, name=f"pw{wmi}", tag="cps")
                nc.tensor.matmul(
                    pw[:],
                    zwarm[:, :, 0:P],
                    zwarm[:],
                    start=True,
                    stop=True,
                    perf_mode=DR,
                )

            # Startup latency: the first conv0 matmul needs xt0 + wdr[0].
            # Those transfers go first, split across four trigger queues.
            xt0 = [
                lpool.tile([P, 2, T], fp8, name=f"xt{kt}", tag=f"xt{kt}")
                for kt in range(DT)
            ]
            for kt in range(DT):
                nc.sync.dma_start(xt0[kt][:], xin[kt, :, :, 0:T])
            wdr = [
                [cpool.tile([P, 2, C], fp8, name=f"wdr{i}_{kt}") for kt in range(DT)]
                for i in range(4)
            ]
            nc.scalar.dma_start(wdr[0][0][:], wdr_d[0, 0])
            nc.gpsimd.dma_start(wdr[0][1][:], wdr_d[0, 1])
            sbc = cpool.tile([P, 28], f32, name="sbc_t")
            nc.scalar.dma_start(sbc[:], sbc_d[:])
            for kt in range(DT):
                nc.scalar.dma_start(wdr[1][kt][:], wdr_d[1, kt])
            onesdr = cpool.tile([P, 2, P], fp8, name="onesdr_t")
            nc.scalar.dma_start(onesdr[:], onesdr_d[:])
            # proxy path + folded-value inputs (needed from ~20us in)
            p_dr = [cpool.tile([P, 2, K], fp8, name=f"pdr{kt}") for kt in range(DT)]
            for kt in range(DT):
                nc.gpsimd.dma_start(p_dr[kt][:], pin_dr[kt])
            for i in range(2, 4):
                for kt in range(DT):
                    nc.gpsimd.dma_start(wdr[i][kt][:], wdr_d[i, kt])
            p_t = [cpool.tile([P, K], bf16, name=f"p{c}") for c in range(CT)]
            for c in range(CT):
                nc.gpsimd.dma_start(p_t[c][:], pin[c * P : (c + 1) * P, :])
            w = [
                [cpool.tile([P, C], bf16, name=f"w{i}_{c}") for c in range(CT)]
                for i in range(3)
            ]
            for i in range(3):  # dram layers 4, 5, 6
                for c in range(CT):
                    nc.gpsimd.dma_start(w[i][c][:], wt[i, c * P : (c + 1) * P, :])
            b6bc = cpool.tile([P, C], f32, name="b6bc_t")
            nc.gpsimd.dma_start(b6bc[:], b6bc_d[:])

            def bias_ap(i, o):
                return sbc[:, i * 4 + o : i * 4 + o + 1]

            def _relu_evict(dst, ps, wi, o):
                # 3 of 4 conv evictions on ACT, 1 on DVE: keeps both
                # engines under the PE streaming time. (GPSIMD cannot
                # read PSUM, so it cannot help here.)
                if o != 3:
                    nc.scalar.activation(dst, ps[:], AF.Relu, bias=bias_ap(wi, o))
                else:
                    nc.vector.tensor_scalar(
                        out=dst,
                        in0=ps[:],
                        scalar1=bias_ap(wi, o),
                        scalar2=0.0,
                        op0=mybir.AluOpType.add,
                        op1=mybir.AluOpType.max,
                    )

            def conv_dr_k(inp, wi, evict):
                """fp8 DoubleRow conv over the K proxy columns.

                inp = DT tiles [P, 2, K]; evict(o, ps) consumes [P, K]."""
                for o in range(CT):
                    ps = psp.tile([P, K], f32, name=f"psk_{wi}_{o}", tag="cps")
                    for kt in range(DT):
                        nc.tensor.matmul(
                            ps[:],
                            wdr[wi][kt][:, :, o * P : (o + 1) * P],
                            inp[kt][:],
                            start=(kt == 0),
                            stop=(kt == DT - 1),
                            perf_mode=DR,
                        )
                    evict(o, ps)

            def conv_bf(inp, wi, evict):
                """bf16 conv over K columns: value path. wi indexes w (0->layer4)."""
                for o in range(CT):
                    ps = psp.tile([P, K], f32, name=f"psb_{wi}_{o}", tag="cps")
                    for c in range(CT):
                        nc.tensor.matmul(
                            ps[:],
                            w[wi][c][:, o * P : (o + 1) * P],
                            inp[c][:],
                            start=(c == 0),
                            stop=(c == CT - 1),
                        )
                    evict(o, ps)

            def conv_dr(inp, wi, evict):
                """fp8 DoubleRow conv over pixel tiles: inp = DT tiles [P,2,T].

                evict(o, ps) consumes the [P, T] psum of output block o."""
                for o in range(CT):
                    ps = psp.tile([P, T], f32, name=f"psd_{wi}_{o}", tag="cps")
                    for kt in range(DT):
                        nc.tensor.matmul(
                            ps[:],
                            wdr[wi][kt][:, :, o * P : (o + 1) * P],
                            inp[kt][:],
                            start=(kt == 0),
                            stop=(kt == DT - 1),
                            perf_mode=DR,
                        )
                    evict(o, ps)

            # ---- main pipeline over pixel tiles ----
            def stage_conv0(t, xt=None):
                if xt is None:
                    xt = [
                        lpool.tile([P, 2, T], fp8, name=f"xt{kt}", tag=f"xt{kt}")
                        for kt in range(DT)
                    ]
                    for kt in range(DT):
                        nc.sync.dma_start(
                            xt[kt][:], xin[kt, :, :, t * T : (t + 1) * T]
                        )
                t1 = [
                    lpool.tile([P, 2, T], fp8, name=f"t1_{kt}", tag=f"t1{kt}")
                    for kt in range(DT)
                ]
                conv_dr(xt, 0, lambda o, ps: _relu_evict(t1[o // 2][:, o % 2, :], ps, 0, o))
                return t1

            def stage_conv1(t1):
                q_dr = [
                    lpool.tile([P, 2, T], fp8, name=f"qdr{kt}", tag=f"qdr{kt}")
                    for kt in range(DT)
                ]
                conv_dr(t1, 1, lambda o, ps: _relu_evict(q_dr[o // 2][:, o % 2, :], ps, 1, o))
                return q_dr

            def sim_block(q_dr, probT, k):
                ps = psp.tile([P, T], f32, name=f"ps_simT{k}", tag="cps")
                for kt in range(DT):
                    nc.tensor.matmul(
                        ps[:],
                        key_dr[kt][:, :, k * P : (k + 1) * P],
                        q_dr[kt][:],
                        start=(kt == 0),
                        stop=(kt == DT - 1),
                        perf_mode=DR,
                    )
                nc.scalar.activation(probT[:, k, :], ps[:], AF.Exp, scale=ALPHA)

            def conv_block(inp, wi, o, dst):
                ps = psp.tile([P, T], f32, name=f"psd_{wi}_{o}", tag="cps")
                for kt in range(DT):
                    nc.tensor.matmul(
                        ps[:],
                        wdr[wi][kt][:, :, o * P : (o + 1) * P],
                        inp[kt][:],
                        start=(kt == 0),
                        stop=(kt == DT - 1),
                        perf_mode=DR,
                    )
                _relu_evict(dst[o // 2][:, o % 2, :], ps, wi, o)

            def b_rs(probT, sl):
                """rowsum matmul + reciprocal for probT columns sl."""
                n_ = sl.stop - sl.start
                ps_rs = psa.tile([P, n_], f32, name="ps_rs", tag="cps")
                nc.tensor.matmul(
                    ps_rs[:],
                    onesdr[:],
                    probT[:, :, sl],
                    start=True,
                    stop=True,
                    perf_mode=DR,
                )
                rc = lpool.tile([P, n_], f32, name="rc", tag="rc")
                nc.vector.reciprocal_approx_fast(out=rc[:], in_=ps_rs[:])
                return rc

            def b_out(probT, sl, o, rc, outt):
                """output block o contraction + normalize-evict."""
                n_ = sl.stop - sl.start
                ps = psa.tile([P, n_], f32, name=f"ps_out{o}", tag="cps")
                nc.tensor.matmul(
                    ps[:],
                    v2T_dr[:, :, o * P : (o + 1) * P],
                    probT[:, :, sl],
                    start=True,
                    stop=True,
                    perf_mode=DR,
                )
                # out = relu(acc * rc) == max(acc, 0) * rc  (rc > 0)
                nc.vector.scalar_tensor_tensor(
                    out=outt[:, o, sl],
                    in0=ps[:],
                    scalar=0.0,
                    in1=rc[:],
                    op0=mybir.AluOpType.max,
                    op1=mybir.AluOpType.mult,
                )

            def stage_b(t, probT, halves=1):
                """Standalone normalize+contract for tile t (used off the
                steady-state path). halves>1 shortens the final DVE chain."""
                outt = lpool.tile([P, CT, T], bf16, name="ot", tag="ot")
                hw_ = T // halves
                for h in range(halves):
                    sl = slice(h * hw_, (h + 1) * hw_)
                    rc = b_rs(probT, sl)
                    for o in range(CT):
                        b_out(probT, sl, o, rc, outt)
                    # per-half DMA: the first half's write overlaps the
                    # second half's compute, shortening the run tail
                    nc.sync.dma_start(
                        out_d[:, :, t * T + sl.start : t * T + sl.stop],
                        outt[:, :, sl],
                    )

            q0 = stage_conv1(stage_conv0(0, xt=xt0))

            # ---- setup: key / folded-value from proxy ----
            # key is evicted as fp8 in the kt-paired DoubleRow layout so the
            # similarity matmul q^T key runs DoubleRow too.
            key_dr = [cpool.tile([P, 2, K], fp8, name=f"keydr{kt}") for kt in range(DT)]
            v2T_dr = cpool.tile([P, 2, C], fp8, name="v2Tdr")
            with tc.tile_pool(name="setup", bufs=1) as spool:
                def _dr_evict(dst_tiles, layer):
                    return lambda o, ps: _relu_evict(
                        dst_tiles[o // 2][:, o % 2, :], ps, layer, o
                    )

                k1 = [spool.tile([P, 2, K], fp8, name=f"k1_{kt}") for kt in range(DT)]
                conv_dr_k(p_dr, 2, _dr_evict(k1, 2))
                conv_dr_k(k1, 3, _dr_evict(key_dr, 3))
                v1 = [spool.tile([P, K], bf16, name=f"v1_{c}") for c in range(CT)]
                conv_bf(
                    p_t,
                    0,
                    lambda o, ps: nc.scalar.activation(
                        v1[o][:], ps[:], AF.Relu, bias=bias_ap(4, o)
                    ),
                )
                val = [spool.tile([P, K], bf16, name=f"val{c}") for c in range(CT)]
                conv_bf(
                    v1,
                    1,
                    lambda o, ps: nc.scalar.activation(
                        val[o][:], ps[:], AF.Relu, bias=bias_ap(5, o)
                    ),
                )
                # v2T[k] = val^T @ (s6 W6)^T + b6 row  (i.e. V2^T blocks)
                for k in range(KT):
                    pt = psa.tile([P, C], f32, name=f"ptv{k}", tag="cps")
                    for c in range(CT):
                        nc.tensor.matmul(
                            pt[:],
                            val[c][:, k * P : (k + 1) * P],
                            w[2][c][:],
                            start=(c == 0),
                            stop=(c == CT - 1),
                        )
                    nc.vector.tensor_tensor(
                        out=v2T_dr[:, k, :],
                        in0=pt[:],
                        in1=b6bc[:],
                        op=mybir.AluOpType.add,
                    )


            # Steady-state iteration t interleaves stage_b(t-1)'s rowsum/out
            # matmuls through conv0(t+1)/sim(t)/conv1(t+1) so their DVE
            # evictions become ready spread across the tile period instead of
            # bursting at its end (which stalled the PE on PSUM WAR). A
            # cost-free 16th PSUM allocation per tile keeps the bank<->stage
            # mapping periodic (16 allocs over 8 banks).
            def xt_fetch(t):
                xt = [
                    lpool.tile([P, 2, T], fp8, name=f"xt{kt}", tag=f"xt{kt}")
                    for kt in range(DT)
                ]
                for kt in range(DT):
                    nc.sync.dma_start(xt[kt][:], xin[kt, :, :, t * T : (t + 1) * T])
                return xt

            FULL = slice(0, T)
            prev = None
            qcur = q0
            # prefetch tile 1's x two iterations ahead of its conv0 use: the
            # interleaved loop reads xt(t+1) right at the iteration top, so
            # its DMA must be issued in iteration t-1, not t.
            xt_next = xt_fetch(1) if n_tiles > 1 else None
            for t in range(n_tiles):
                last = t + 1 >= n_tiles
                if not last:
                    xt = xt_next
                    xt_next = xt_fetch(t + 2) if t + 2 < n_tiles else None
                    t1 = [
                        lpool.tile([P, 2, T], fp8, name=f"t1_{kt}", tag=f"t1{kt}")
                        for kt in range(DT)
                    ]
                    q_n = [
                        lpool.tile([P, 2, T], fp8, name=f"qdr{kt}", tag=f"qdr{kt}")
                        for kt in range(DT)
                    ]
                probT = lpool.tile([P, 2, T], fp8, name="pT", tag="pT")
                if prev is not None:
                    outt = lpool.tile([P, CT, T], bf16, name="ot", tag="ot")

                # Emission order (PSUM alloc indices 0..15; bank = idx mod 8):
                # [c0_0 c0_1 rs c0_2 c0_3 sk0 out0 PAD c1_0 sk1 out1 c1_1
                #  out2 c1_2 out3 c1_3]. The pad at idx 7 pairs the latest
                # DVE eviction (c1o3's) with a bank nothing writes, and every
                # other (idx, idx+8) pair has >=1.3us eviction->reuse margin.
                if not last:
                    conv_block(xt, 0, 0, t1)
                    conv_block(xt, 0, 1, t1)
                if prev is not None:
                    rc = b_rs(prev[1], FULL)
                if not last:
                    conv_block(xt, 0, 2, t1)
                    conv_block(xt, 0, 3, t1)
                sim_block(qcur, probT, 0)
                if prev is not None:
                    b_out(prev[1], FULL, 0, rc, outt)
                    psa.tile([P, T], f32, name="ps_pad", tag="cps")
                if not last:
                    conv_block(t1, 1, 0, q_n)
                sim_block(qcur, probT, 1)
                if prev is not None:
                    b_out(prev[1], FULL, 1, rc, outt)
                if not last:
                    conv_block(t1, 1, 1, q_n)
                if prev is not None:
                    b_out(prev[1], FULL, 2, rc, outt)
                if not last:
                    conv_block(t1, 1, 2, q_n)
                if prev is not None:
                    b_out(prev[1], FULL, 3, rc, outt)
                    nc.sync.dma_start(
                        out_d[:, :, prev[0] * T : (prev[0] + 1) * T], outt[:]
                    )
                if not last:
                    conv_block(t1, 1, 3, q_n)
                    qcur = q_n
                prev = (t, probT)
            stage_b(prev[0], prev[1], halves=2)

    nc.compile()
    return nc


def make_in_maps(x, proxy, W, s, b):
    # s > 0, so relu(s*(W@x)+b) == relu((diag(s)W)@x + b): fold s into W.
    w_eff = s[:, :, None].astype(np.float64) * W.astype(np.float64)
    wt_full = np.ascontiguousarray(w_eff.transpose(0, 2, 1))  # [7, c, o]
    # DoubleRow interleaved fp8 weights for layers 0-3:
    # wdr[i, kt, p, j, o] = wt[i, kt*256 + j*128 + p, o]
    wdr = np.ascontiguousarray(
        wt_full[:4].reshape(4, DT, 2, P, C).transpose(0, 1, 3, 2, 4)
    ).astype(NP_FP8)
    wt = np.ascontiguousarray(wt_full[4:]).astype(NP_BF16)  # layers 4..6
    sbc = np.ascontiguousarray(
        b.reshape(7, CT, P).transpose(2, 0, 1).reshape(P, 7 * CT)
    ).astype(np.float32)
    b6bc = np.broadcast_to(b[6].astype(np.float32)[None, :], (P, C)).copy()
    onesdr = np.ones((P, 2, P), dtype=NP_FP8)
    in_maps = []
    for n in range(N):
        # xin[kt, p, j, col] = x[n, kt*256 + j*128 + p, col]
        xin = np.ascontiguousarray(
            x[n].reshape(DT, 2, P, HW).transpose(0, 2, 1, 3)
        ).astype(NP_FP8)
        pf = proxy[n].reshape(C, K)
        pin_dr = np.ascontiguousarray(
            pf.reshape(DT, 2, P, K).transpose(0, 2, 1, 3)
        ).astype(NP_FP8)
        in_maps.append(
            {
                "xin": xin,
                "pindr": pin_dr,
                "pin": np.ascontiguousarray(pf).astype(NP_BF16),
                "wdr": wdr,
                "wt": wt,
                "sbc": sbc,
                "b6bc": b6bc,
                "onesdr": onesdr,
            }
        )
    return in_maps


_CACHED = {}


def _get_module():
    if "nc" not in _CACHED:
        _CACHED["nc"] = build_module()
    return _CACHED["nc"]


def kernel(x, proxy, W, s, b):
    nc = _get_module()
    in_maps = make_in_maps(x, proxy, W, s, b)
    res = bass_utils.run_bass_kernel_spmd(nc, in_maps, core_ids=list(range(N)))
    out = np.stack(
        [
            np.ascontiguousarray(
                res.results[n]["out"].transpose(1, 0, 2)
            ).reshape(C, H, W_IMG)
            for n in range(N)
        ]
    )
    return out.astype(np.float32)
